# revision 31
# baseline (speedup 1.0000x reference)
"""BiaffineDependencyParser energy kernel for 8 Trainium2 NeuronCores.

Data-parallel over the batch: each of the 8 cores computes the biaffine
edge energies for 4 of the 32 examples entirely in fp32; the (inherently
sequential, per-example) Chu-Liu-Edmonds decode runs on the host from the
device-computed energies.

Device layout notes (per core, B_local=4, T=128, D=768, A=512, R=128, L=50):
  - everything is computed "transposed" (dep token j on partitions) so both
    softmax reductions are free-dim reductions:
      projT[a, (b t)] = W.T @ X.T           (bias+ELU fused into evacuation)
      s1T[c, (b t)]   = U_arc.T @ h_arcT    (contraction over a)
      arcT_b[j, i]    = d_arcT_b.T @ s1T_b  (contraction over c)
      M[s, (l b i)]   = U_lab[l].T @ h_labT (contraction over r)
      labT_b[j,(i l)] = d_labT_b.T @ M_b    (contraction over s, l innermost)
  - arc softmax over heads i and lab softmax over labels l both become
    per-partition-row reductions (ACT accum_out / strided DVE reduce).
  - energy is emitted as [b, j, i, l] and transposed to [b, l, i, j] on host.
"""

import sys

sys.path.insert(0, '/opt/trn_rl_repo')

import numpy as np
from contextlib import ExitStack

B, T, D, A, R, L = 32, 128, 768, 512, 128, 50
NCORES = 8
BL = B // NCORES          # 4 examples per core
DC = D // 128             # 6 contraction chunks
AC = A // 128             # 4 A-chunks
IC_SIZES = [10] * 12 + [8]  # i-group sizes for stage 3b (10*50=500 <= 512)

_PROGRAM = None


def _build_program():
    import concourse.bass as bass
    import concourse.tile as tile
    import concourse.mybir as mybir
    from concourse import bacc, masks

    dt = mybir.dt
    AF = mybir.ActivationFunctionType
    ALU = mybir.AluOpType

    nc = bacc.Bacc(trn_type="TRN2", target_bir_lowering=False, debug=False)

    xt_r = nc.dram_tensor("xt_r", [D, BL * T], dt.float32, kind="ExternalInput").ap()
    xt_e = nc.dram_tensor("xt_e", [D, BL * T], dt.float32, kind="ExternalInput").ap()
    w_ha_r = nc.dram_tensor("w_ha_r", [D, A], dt.float32, kind="ExternalInput").ap()
    w_ha_e = nc.dram_tensor("w_ha_e", [D, A], dt.float32, kind="ExternalInput").ap()
    b_ha = nc.dram_tensor("b_ha", [A], dt.float32, kind="ExternalInput").ap()
    w_da_r = nc.dram_tensor("w_da_r", [D, A], dt.float32, kind="ExternalInput").ap()
    w_da_e = nc.dram_tensor("w_da_e", [D, A], dt.float32, kind="ExternalInput").ap()
    b_da = nc.dram_tensor("b_da", [A], dt.float32, kind="ExternalInput").ap()
    u_arc = nc.dram_tensor("u_arc", [A, A], dt.float32, kind="ExternalInput").ap()
    w_hl_r = nc.dram_tensor("w_hl_r", [D, R], dt.float32, kind="ExternalInput").ap()
    w_hl_e = nc.dram_tensor("w_hl_e", [D, R], dt.float32, kind="ExternalInput").ap()
    b_hl = nc.dram_tensor("b_hl", [R], dt.float32, kind="ExternalInput").ap()
    w_dl_r = nc.dram_tensor("w_dl_r", [D, R], dt.float32, kind="ExternalInput").ap()
    w_dl_e = nc.dram_tensor("w_dl_e", [D, R], dt.float32, kind="ExternalInput").ap()
    b_dl = nc.dram_tensor("b_dl", [R], dt.float32, kind="ExternalInput").ap()
    u_lab_r = nc.dram_tensor("u_lab_r", [L, R, R], dt.float32, kind="ExternalInput").ap()
    u_lab_e = nc.dram_tensor("u_lab_e", [L, R, R], dt.float32, kind="ExternalInput").ap()

    # energy, transposed: [b, j(dep), i(head), l(label)]
    out_e = nc.dram_tensor("energy_t", [BL, T, T, L], dt.float32,
                           kind="ExternalOutput").ap()

    with tile.TileContext(nc) as tc, ExitStack() as ctx:
        # Pools are LIFO stack-allocated: open in reverse order of close.
        const = ctx.enter_context(tc.tile_pool(name="const", bufs=1))
        small = ctx.enter_context(tc.tile_pool(name="small", bufs=1))
        pool_dl = ctx.enter_context(tc.tile_pool(name="pool_dl", bufs=1))
        pool_hl = ctx.enter_context(tc.tile_pool(name="pool_hl", bufs=1))
        s_arc = ExitStack()
        pool_arc = s_arc.enter_context(tc.tile_pool(name="pool_arc", bufs=1))
        s_w = ExitStack()
        pool_w = s_w.enter_context(tc.tile_pool(name="pool_w", bufs=1))
        pmm = ctx.enter_context(tc.tile_pool(name="pmm", bufs=4, space="PSUM"))
        plab = ctx.enter_context(tc.tile_pool(name="plab", bufs=2, space="PSUM"))

        # ---------------- input DMAs ----------------



        # ---------------- stage 0: load pre-transposed X ----------------
        # xT[d, (b t)] pair, already transposed and f32r-split on the host
        xT = pool_w.tile([128, DC * BL * T], dt.float32r, tag="xT")
        xTe = pool_w.tile([128, DC * BL * T], dt.float32r, tag="xTe")
        xTv = xT[:].rearrange("p (c n) -> p c n", c=DC, n=BL * T)
        xTev = xTe[:].rearrange("p (c n) -> p c n", c=DC, n=BL * T)
        xtr_v = xt_r.rearrange("(c p) n -> c p n", p=128)
        xte_v = xt_e.rearrange("(c p) n -> c p n", p=128)
        wha_t = pool_w.tile([128, DC * A], dt.float32r, tag="wha")
        wha_et = pool_w.tile([128, DC * A], dt.float32r, tag="wha_e")
        wda_t = pool_w.tile([128, DC * A], dt.float32r, tag="wda")
        wda_et = pool_w.tile([128, DC * A], dt.float32r, tag="wda_e")
        whl_t = pool_w.tile([128, DC * R], dt.float32r, tag="whl")
        whl_et = pool_w.tile([128, DC * R], dt.float32r, tag="whl_e")
        wdl_t = pool_w.tile([128, DC * R], dt.float32r, tag="wdl")
        wdl_et = pool_w.tile([128, DC * R], dt.float32r, tag="wdl_e")

        cast_n = [0]

        def load_cast_chunk(dst, src_ap, eng=None):
            # DMA one chunk into an f32 staging tile, then cast into the f32r
            # tile: the verifier requires every writer of f32r-consumed
            # memory to be a rounding compute instruction, so the DMA must
            # not target the f32r tile itself. Host data is pre-rounded, so
            # the cast changes no values.
            n = dst.free_size()
            stg = pool_w.tile([128, 640], dt.float32, tag="stg", bufs=6,
                              name=f"stg{cast_n[0]}")
            nc.sync.dma_start(stg[:, 0:n], src_ap)
            if eng is not None:
                eng.tensor_copy(dst, stg[:, 0:n])
            else:
                k = cast_n[0] % 3
                if k == 0:
                    nc.vector.tensor_copy(dst, stg[:, 0:n])
                elif k == 1:
                    nc.scalar.copy(dst, stg[:, 0:n])
                else:
                    nc.gpsimd.tensor_copy(dst, stg[:, 0:n])
            cast_n[0] += 1

        wspecs = (("wha", wha_t, w_ha_r, A), ("wha_e", wha_et, w_ha_e, A),
                  ("wda", wda_t, w_da_r, A), ("wda_e", wda_et, w_da_e, A),
                  ("whl", whl_t, w_hl_r, R), ("whl_e", whl_et, w_hl_e, R),
                  ("wdl", wdl_t, w_dl_r, R), ("wdl_e", wdl_et, w_dl_e, R))
        wviews = {nm: (t_[:].rearrange("p (c a) -> p c a", c=DC, a=w),
                       ap_.rearrange("(c p) a -> c p a", p=128))
                  for nm, t_, ap_, w in wspecs}

        # interleaved in first-use order: per chunk c, x pair then W_head_arc
        # pair; remaining weights afterwards, chunked
        for c in range(DC):
            load_cast_chunk(xTv[:, c, :], xtr_v[c])
            load_cast_chunk(xTev[:, c, :], xte_v[c])
            for nm in ("wha", "wha_e"):
                tv, av = wviews[nm]
                load_cast_chunk(tv[:, c, :], av[c])
        bha_t = small.tile([128, AC], dt.float32, tag="bha")
        bda_t = small.tile([128, AC], dt.float32, tag="bda")
        bhl_t = small.tile([128, 1], dt.float32, tag="bhl")
        bdl_t = small.tile([128, 1], dt.float32, tag="bdl")
        nc.sync.dma_start(bha_t[:], b_ha.rearrange("(c p) -> p c", p=128))
        nc.sync.dma_start(bda_t[:], b_da.rearrange("(c p) -> p c", p=128))
        nc.sync.dma_start(bhl_t[:], b_hl.rearrange("(c p) -> p c", p=128))
        nc.sync.dma_start(bdl_t[:], b_dl.rearrange("(c p) -> p c", p=128))

        for nm in ("wda", "wda_e", "whl", "whl_e", "wdl", "wdl_e"):
            tv, av = wviews[nm]
            for c in range(DC):
                load_cast_chunk(tv[:, c, :], av[c])

        ua_t = pool_arc.tile([128, AC * A], dt.float32, tag="ua")
        nc.sync.dma_start(
            ua_t[:].rearrange("p (c a) -> p c a", c=AC, a=A),
            u_arc.rearrange("(c p) a -> p c a", p=128))

        ul_t = pool_dl.tile([128, L * R], dt.float32r, tag="ul")
        ul_et = pool_dl.tile([128, L * R], dt.float32r, tag="ul_e")
        # [p, l, s] view of the DRAM source, chunked by groups of 5 labels
        ulr_v = u_lab_r.rearrange("l p s -> p l s")
        ule_v = u_lab_e.rearrange("l p s -> p l s")
        for t_, sv in ((ul_t, ulr_v), (ul_et, ule_v)):
            for q in range(10):
                sl = t_[:, q * 640:(q + 1) * 640]
                load_cast_chunk(
                    sl.rearrange("p (l s) -> p l s", l=5, s=R),
                    sv[:, q * 5:(q + 1) * 5, :], eng=nc.gpsimd)


        # ---------------- stage 1: projections (transposed) + ELU ------
        # projT tiles: [128, n_chunks * (BL*T)]
        h_arcT = pool_arc.tile([128, AC * BL * T], dt.float32, tag="h_arcT")
        d_arcT = pool_arc.tile([128, AC * BL * T], dt.float32, tag="d_arcT")
        h_labT = pool_hl.tile([128, BL * T], dt.float32, tag="h_labT")
        d_labT = pool_dl.tile([128, BL * T], dt.float32, tag="d_labT")

        NBT = BL * T

        def elu_evac(psum_ap, bias_ap, out_ap, tmp_pool):
            # out = elu(psum + bias) = min(exp(z) - 1, relu(z)), z = psum + bias
            e_t = tmp_pool.tile([128, NBT], dt.float32, tag="elu_e")
            r_t = tmp_pool.tile([128, NBT], dt.float32, tag="elu_r")
            nc.scalar.activation(e_t[:], psum_ap, AF.Exp, bias=bias_ap)
            nc.vector.tensor_scalar(r_t[:], psum_ap, bias_ap, 0.0,
                                    op0=ALU.add, op1=ALU.max)
            nc.vector.scalar_tensor_tensor(out_ap, e_t[:], -1.0, r_t[:],
                                           op0=ALU.add, op1=ALU.min)

        with tc.tile_pool(name="elu_tmp", bufs=2) as etmp:
            specs = ((wha_t, wha_et, bha_t, AC, A, h_arcT),
                     (wda_t, wda_et, bda_t, AC, A, d_arcT),
                     (whl_t, whl_et, bhl_t, 1, R, h_labT),
                     (wdl_t, wdl_et, bdl_t, 1, R, d_labT))
            for (w_t, w_et, bias_t, nchunks, width, out_t) in specs:
                wv = w_t[:].rearrange("p (c a) -> p c a", c=DC, a=width)
                wev = w_et[:].rearrange("p (c a) -> p c a", c=DC, a=width)
                for ac in range(nchunks):
                    pp = pmm.tile([128, NBT], dt.float32, tag="mm")
                    asl = slice(ac * 128, (ac + 1) * 128)
                    for c in range(DC):
                        nc.tensor.matmul(pp[:], wv[:, c, asl], xTv[:, c, :],
                                         start=(c == 0), stop=False)
                        nc.tensor.matmul(pp[:], wev[:, c, asl], xTv[:, c, :],
                                         start=False, stop=False)
                        nc.tensor.matmul(pp[:], wv[:, c, asl], xTev[:, c, :],
                                         start=False, stop=(c == DC - 1))
                    elu_evac(pp[:], bias_t[:, ac:ac + 1],
                             out_t[:, ac * NBT:(ac + 1) * NBT]
                             if nchunks > 1 else out_t[:, :],
                             etmp)

        s_w.close()

        # ---------------- stage 2: arc scores ----------------
        s1T = pool_arc.tile([128, AC * NBT], dt.float32, tag="s1T")
        uav = ua_t[:].rearrange("p (c a) -> p c a", c=AC, a=A)
        hav = h_arcT[:].rearrange("p (c n) -> p c n", c=AC, n=NBT)
        dav = d_arcT[:].rearrange("p (c n) -> p c n", c=AC, n=NBT)
        s1v = s1T[:].rearrange("p (c n) -> p c n", c=AC, n=NBT)
        g_t = small.tile([128, BL * T], dt.float32, tag="g")       # exp(arcT)
        rs_t = small.tile([128, BL], dt.float32, tag="rs")         # 1/sum_i
        sa_t = small.tile([128, BL], dt.float32, tag="sa")

        if True:
            for cc in range(AC):
                pp = pmm.tile([128, NBT], dt.float32, tag="mm")
                for ac in range(AC):
                    nc.tensor.matmul(
                        pp[:], uav[:, ac, cc * 128:(cc + 1) * 128],
                        hav[:, ac, :], start=(ac == 0), stop=(ac == AC - 1))
                nc.scalar.copy(s1v[:, cc, :], pp[:])

            for b in range(BL):
                pa = pmm.tile([128, 512], dt.float32, tag="mm")
                pa = pa[0:128, 0:128]
                for cc in range(AC):
                    nc.tensor.matmul(
                        pa, dav[:, cc, b * T:(b + 1) * T],
                        s1v[:, cc, b * T:(b + 1) * T],
                        start=(cc == 0), stop=(cc == AC - 1))
                # g = exp(arcT), sa = sum_i exp(arcT)  (heads softmax denom)
                nc.scalar.activation(g_t[:, b * T:(b + 1) * T], pa,
                                     AF.Exp, accum_out=sa_t[:, b:b + 1])
            nc.vector.reciprocal(rs_t[:], sa_t[:])

        s_arc.close()
        pool_m = ctx.enter_context(tc.tile_pool(name="pool_m", bufs=1))

        # ---------------- stage 3a: M = U_lab[l].T @ h_labT -------------
        # h_labT split into an f32r (value, residual) pair
        hlr = pool_hl.tile([128, NBT], dt.float32r, tag="hlr")
        hle = pool_hl.tile([128, NBT], dt.float32r, tag="hle")
        nc.vector.tensor_copy(hlr[:], h_labT[:])
        nc.vector.scalar_tensor_tensor(
            hle[:], h_labT[:], 0.0, hlr[:].bitcast(dt.float32),
            op0=ALU.add, op1=ALU.subtract)

        m_t = pool_m.tile([128, L * NBT], dt.float32, tag="m")  # [s, (l b i)]
        mv = m_t[:].rearrange("p (l n) -> p l n", l=L, n=NBT)
        ulv = ul_t[:].rearrange("p (l s) -> p l s", l=L, s=R)
        ulev = ul_et[:].rearrange("p (l s) -> p l s", l=L, s=R)
        for l in range(L):
            pp = pmm.tile([128, NBT], dt.float32, tag="mm")
            nc.tensor.matmul(pp[:], ulv[:, l, :], hlr[:],
                             start=True, stop=False)
            nc.tensor.matmul(pp[:], ulev[:, l, :], hlr[:],
                             start=False, stop=False)
            nc.tensor.matmul(pp[:], ulv[:, l, :], hle[:],
                             start=False, stop=True)
            if l % 4 == 0:
                nc.scalar.copy(mv[:, l, :], pp[:])
            else:
                nc.vector.tensor_copy(mv[:, l, :], pp[:])
        epool = ctx.enter_context(tc.tile_pool(name="epool", bufs=8))

        # ---------------- stage 3b: lab scores, E, energy ----------------
        # Per example b: 13 i-groups of (10 or 8) heads, 2 groups per
        # [128,1024] psum tile (2 banks). Post-processing
        # (exp -> S -> W -> mul -> DMA) pipelined per psum tile.
        with tc.tile_pool(name="swpool", bufs=8) as swpool:
            groups = []
            for b in range(BL):
                i0 = 0
                k = 0
                while k < len(IC_SIZES):
                    ks = list(range(k, min(k + 2, len(IC_SIZES))))
                    groups.append((b, i0, ks))
                    i0 += sum(IC_SIZES[kk] for kk in ks)
                    k += len(ks)
            ngroups = len(groups)
            for gi, (b, i0, ks) in enumerate(groups):
                rhs_all = mv[:, :, b * T:(b + 1) * T].transpose([0, 2, 1])
                icnt = sum(IC_SIZES[kk] for kk in ks)
                pt = plab.tile([128, 1024], dt.float32, tag="labp")
                ib = i0
                for idx, kk in enumerate(ks):
                    isz = IC_SIZES[kk]
                    nc.tensor.matmul(
                        pt[:, idx * 512:idx * 512 + isz * L],
                        d_labT[:, b * T:(b + 1) * T],
                        rhs_all[:, ib:ib + isz, :],
                        start=True, stop=True)
                    ib += isz
                # exp over the used columns of the psum tile
                nval = IC_SIZES[ks[0]] * L
                e_t = epool.tile([128, 1024], dt.float32, tag="E")
                pv = pt[:].rearrange("p (c n) -> p c n", c=2, n=512)[
                    :, 0:len(ks), 0:nval]
                ev = e_t[:, 0:icnt * L].rearrange(
                    "p (c n) -> p c n", c=len(ks), n=nval)
                nc.scalar.activation(ev, pv, AF.Exp)
                # S over labels, W = exp(arc) * rs / S, energy = E * W
                ev3 = e_t[:, 0:icnt * L].rearrange(
                    "p (i l) -> p i l", i=icnt, l=L)
                s_t = swpool.tile([128, 20], dt.float32, tag="S")
                w_t = swpool.tile([128, 20], dt.float32, tag="W")
                nc.vector.tensor_reduce(
                    s_t[:, 0:icnt], ev3, axis=mybir.AxisListType.X,
                    op=mybir.AluOpType.add)
                nc.vector.reciprocal(w_t[:, 0:icnt], s_t[:, 0:icnt])
                nc.vector.scalar_tensor_tensor(
                    w_t[:, 0:icnt], g_t[:, b * T + i0:b * T + i0 + icnt],
                    rs_t[:, b:b + 1], w_t[:, 0:icnt],
                    op0=mybir.AluOpType.mult, op1=mybir.AluOpType.mult)
                wb = w_t[:, 0:icnt].unsqueeze(2).broadcast_to([128, icnt, L])
                if gi % 2 == 0 and gi < ngroups - 4:
                    nc.gpsimd.tensor_tensor(ev3, ev3, wb,
                                            op=mybir.AluOpType.mult)
                else:
                    nc.vector.tensor_tensor(ev3, ev3, wb,
                                            op=mybir.AluOpType.mult)
                nc.sync.dma_start(
                    out_e[b].rearrange("j i l -> j (i l)")[
                        :, i0 * L:(i0 + icnt) * L],
                    e_t[:, 0:icnt * L])

    nc.compile()
    return nc


def _get_program():
    global _PROGRAM
    if _PROGRAM is None:
        _PROGRAM = _build_program()
    return _PROGRAM


# ---------------- Chu-Liu-Edmonds MST decode (verbatim host port) ----------


def _find_cycle(parents, length, current_nodes):
    added = [False] * length
    added[0] = True
    cycle = set()
    has_cycle = False
    for i in range(1, length):
        if has_cycle:
            break
        if added[i] or not current_nodes[i]:
            continue
        this_cycle = {i}
        added[i] = True
        has_cycle = True
        next_node = i
        while parents[next_node] not in this_cycle:
            next_node = parents[next_node]
            if added[next_node]:
                has_cycle = False
                break
            added[next_node] = True
            this_cycle.add(next_node)
        if has_cycle:
            original = next_node
            cycle.add(original)
            next_node = parents[original]
            while next_node != original:
                cycle.add(next_node)
                next_node = parents[next_node]
            break
    return has_cycle, list(cycle)


def chu_liu_edmonds(length, score_matrix, current_nodes, final_edges,
                    old_input, old_output, representatives):
    parents = [-1]
    for node1 in range(1, length):
        parents.append(0)
        if current_nodes[node1]:
            max_score = score_matrix[0, node1]
            for node2 in range(1, length):
                if node2 == node1 or not current_nodes[node2]:
                    continue
                new_score = score_matrix[node2, node1]
                if new_score > max_score:
                    max_score = new_score
                    parents[node1] = node2
    has_cycle, cycle = _find_cycle(parents, length, current_nodes)
    if not has_cycle:
        final_edges[0] = -1
        for node in range(1, length):
            if not current_nodes[node]:
                continue
            parent = old_input[parents[node], node]
            child = old_output[parents[node], node]
            final_edges[child] = parent
        return
    cycle_weight = 0.0
    for node in cycle:
        cycle_weight += score_matrix[parents[node], node]
    cycle_representative = cycle[0]
    for node in range(length):
        if not current_nodes[node] or node in cycle:
            continue
        in_edge_weight = float('-inf')
        in_edge = -1
        out_edge_weight = float('-inf')
        out_edge = -1
        for node_in_cycle in cycle:
            if score_matrix[node_in_cycle, node] > in_edge_weight:
                in_edge_weight = score_matrix[node_in_cycle, node]
                in_edge = node_in_cycle
            score = (cycle_weight + score_matrix[node, node_in_cycle]
                     - score_matrix[parents[node_in_cycle], node_in_cycle])
            if score > out_edge_weight:
                out_edge_weight = score
                out_edge = node_in_cycle
        score_matrix[cycle_representative, node] = in_edge_weight
        old_input[cycle_representative, node] = old_input[in_edge, node]
        old_output[cycle_representative, node] = old_output[in_edge, node]
        score_matrix[node, cycle_representative] = out_edge_weight
        old_output[node, cycle_representative] = old_output[node, out_edge]
        old_input[node, cycle_representative] = old_input[node, out_edge]
    considered = []
    for i, node_in_cycle in enumerate(cycle):
        considered.append(set())
        if i > 0:
            current_nodes[node_in_cycle] = False
        for node in representatives[node_in_cycle]:
            considered[i].add(node)
            if i > 0:
                representatives[cycle_representative].add(node)
    chu_liu_edmonds(length, score_matrix, current_nodes, final_edges,
                    old_input, old_output, representatives)
    found = False
    key_node = -1
    for i, node in enumerate(cycle):
        for cycle_rep in considered[i]:
            if cycle_rep in final_edges:
                key_node = node
                found = True
                break
        if found:
            break
    previous = parents[key_node]
    while previous != key_node:
        child = old_output[parents[previous], previous]
        parent = old_input[parents[previous], previous]
        final_edges[child] = parent
        previous = parents[previous]


def decode_mst(energy, length):
    max_length = energy.shape[-1]
    energy = energy[:, :length, :length]
    label_id_matrix = energy.argmax(axis=0)
    score_matrix = np.array(energy.max(axis=0), copy=True)
    old_input = np.zeros([length, length], dtype=np.int32)
    old_output = np.zeros([length, length], dtype=np.int32)
    current_nodes = [True] * length
    representatives = []
    for node1 in range(length):
        score_matrix[node1, node1] = 0.0
        representatives.append({node1})
        for node2 in range(node1 + 1, length):
            old_input[node1, node2] = node1
            old_output[node1, node2] = node2
            old_input[node2, node1] = node2
            old_output[node2, node1] = node1
    final_edges = {}
    chu_liu_edmonds(length, score_matrix, current_nodes, final_edges,
                    old_input, old_output, representatives)
    heads = np.zeros([max_length], np.int32)
    head_type = np.ones([max_length], np.int32)
    for child, parent in final_edges.items():
        heads[child] = parent
        head_type[child] = label_id_matrix[parent, child]
    return heads, head_type


def _decode_one(args):
    energy_i, length_i = args
    return decode_mst(energy_i, length_i)


def _f32r_round(x):
    """Round fp32 to the f32r grid: round-to-nearest-even dropping the low
    12 bits of the fp32 encoding (verified bit-exact against the DVE cast
    on TRN2 hardware)."""
    u = np.ascontiguousarray(x, np.float32).view(np.uint32).astype(np.uint64)
    mask = np.uint64(0xFFF)
    half = np.uint64(0x800)
    low = u & mask
    r = (u + half) & ~mask
    tie = low == half
    odd = ((r >> np.uint64(12)) & np.uint64(1)).astype(bool)
    r = r - np.where(tie & odd, np.uint64(0x1000), np.uint64(0))
    return (r & np.uint64(0xFFFFFFFF)).astype(np.uint32).view(np.float32).reshape(x.shape)


def _f32r_split(x):
    x = np.ascontiguousarray(np.asarray(x, np.float32))
    r = _f32r_round(x)
    e = _f32r_round((x - r).astype(np.float32))
    return r, e


def kernel(encoded_text, W_head_arc, b_head_arc, W_dep_arc, b_dep_arc, U_arc,
           W_head_lab, b_head_lab, W_dep_lab, b_dep_lab, U_lab, mask):
    from concourse.bass_utils import run_bass_kernel_spmd

    nc = _get_program()

    x_r, x_e = _f32r_split(encoded_text)
    w_ha_r, w_ha_e = _f32r_split(W_head_arc)
    w_da_r, w_da_e = _f32r_split(W_dep_arc)
    w_hl_r, w_hl_e = _f32r_split(W_head_lab)
    w_dl_r, w_dl_e = _f32r_split(W_dep_lab)
    u_lab_r, u_lab_e = _f32r_split(U_lab)
    shared = {
        'w_ha_r': w_ha_r, 'w_ha_e': w_ha_e,
        'b_ha': np.ascontiguousarray(np.asarray(b_head_arc, np.float32)),
        'w_da_r': w_da_r, 'w_da_e': w_da_e,
        'b_da': np.ascontiguousarray(np.asarray(b_dep_arc, np.float32)),
        'u_arc': np.ascontiguousarray(np.asarray(U_arc, np.float32)),
        'w_hl_r': w_hl_r, 'w_hl_e': w_hl_e,
        'b_hl': np.ascontiguousarray(np.asarray(b_head_lab, np.float32)),
        'w_dl_r': w_dl_r, 'w_dl_e': w_dl_e,
        'b_dl': np.ascontiguousarray(np.asarray(b_dep_lab, np.float32)),
        'u_lab_r': u_lab_r, 'u_lab_e': u_lab_e,
    }
    core_ids = list(range(NCORES))

    def _xt(a, c):
        shard = a[c * BL:(c + 1) * BL]  # [BL, T, D]
        return np.ascontiguousarray(
            np.transpose(shard, (2, 0, 1)).reshape(D, BL * T))

    in_maps = [dict(shared, xt_r=_xt(x_r, c), xt_e=_xt(x_e, c))
               for c in core_ids]

    res = run_bass_kernel_spmd(nc, in_maps, core_ids)

    # gather: per-core [BL, j, i, l] -> [B, L, T, T]
    energy_t = np.concatenate([res.results[c]['energy_t'] for c in core_ids],
                              axis=0)
    energy = np.ascontiguousarray(np.transpose(energy_t, (0, 3, 2, 1)))

    lengths = np.asarray(mask).sum(-1).astype(np.int64)
    args = [(energy[i], int(lengths[i])) for i in range(B)]
    try:
        import multiprocessing as mp
        with mp.get_context('fork').Pool(min(8, B)) as pool:
            results = pool.map(_decode_one, args)
    except Exception:
        results = [_decode_one(a) for a in args]
    heads = np.stack([r[0] for r in results])
    tags = np.stack([r[1] for r in results])
    return energy, heads, tags
